# revision 28
# baseline (speedup 1.0000x reference)
"""Trainium2 Bass kernel for nn_NeuralGraphHidden (GNN message passing).

Structure: edges ~ randint(-1, 128) makes ~95.5% of atoms degree 6, whose
outputs are exactly zero (the reference's degree mask covers 0..5 only).  Of
the ~1440 "active" atoms, ~99% are degree 5.  The device handles ONLY the
degree-5 atoms (balanced across the 8 cores, NA~184/core); the handful of
degree<5 atoms are computed exactly on the host in numpy (microseconds).

Everything on device is bf16 (f32 PSUM accumulation): halves DMA vs f32,
LDWEIGHTS gets FWL (2x), and matmuls stream 1 col/cycle at any width.
Per-core device pipeline, with edge slots packed so the 5 real edges occupy
slots 0-4 and the padding slot's bond sits in slot 5 (nbr contribution zero):

  msg0_g  = elu(W0a.T @ nbrT_g + W0b.T @ bondT_g)    3 groups of 2 slots
  msg1_g  = elu(W1.T @ msg0_g)
  h0      = elu(iw0lo5.T @ actT + sum_j iw0hi5.T @ msg1_j)  (PSUM accumulate)
  out     = elu(iw15.T @ h0)                          -> bf16 DMA out

elu(x) = relu(x) + min(exp(x), 1) - 1: exp on the ACT engine (bf16 out), the
combine as one fused custom-DVE op.  An ACT-table prewarm and a PE clock-ramp
matmul burst run during the initial DMA wait.
"""

import sys

if "/opt/trn_rl_repo" not in sys.path:
    sys.path.insert(0, "/opt/trn_rl_repo")

import numpy as np
import ml_dtypes

import concourse.bass as bass
import concourse.bacc as bacc
import concourse.mybir as mybir
import concourse.tile as tile
from concourse import bass_utils

import concourse.dve_ops as dve_ops
from concourse.dve_spec import (Spec, Src0, Src1, C0, C1, Zero, maxx, minn,
                                lower)
from concourse.dve_uop import DveOpSpec


def _make_elu_op():
    """out = relu(in0) + min(in1, c0) + c1  -- with c0=1, c1=-1 and
    in1=exp(in0) this is exactly elu(in0)."""
    name = "ELU_FUSED_ANT"
    for op in dve_ops.OPS:
        if op.name == name:
            return op
    spec = Spec(
        body=maxx(Src0, Zero) + minn(Src1, C0) + C1,
        reference=lambda in0, in1, c0, c1, c2: (
            np.maximum(in0.astype(np.float32), 0)
            + np.minimum(in1.astype(np.float32), c0) + c1),
    )
    idx = dve_ops._CUSTOM_DVE_ROW_BASE + len(dve_ops.OPS)
    shas = {}
    for ver in ("v3", "v4"):
        compiled = DveOpSpec(name=name, opcode=idx, uops=lower(spec, ver=ver),
                             rd1_en=True)
        shas[ver] = compiled.sha(ver)
    op = dve_ops.DveOp(name, spec, subdim=False, uops_sha=shas)
    dve_ops.OPS.append(op)
    dve_ops.CUSTOM_DVE_SPECS[name] = spec
    dve_ops._SUB_OPCODE_FOR_NAME[name] = idx
    return op


ELU_OP = _make_elu_op()

BF16 = ml_dtypes.bfloat16
F32 = mybir.dt.float32
BF = mybir.dt.bfloat16
AF = mybir.ActivationFunctionType
ALU = mybir.AluOpType

B, M, D = 256, 128, 6
FA, FB, MSG, CONV = 128, 32, 128, 128
NCORES = 8

WARMUP_MMS = 5       # PE clock-ramp burst during the initial DMA wait


def _roundup(x, m):
    return (x + m - 1) // m * m


# --------------------------------------------------------------------------
# device program
# --------------------------------------------------------------------------

def build_program(NA, warmup=WARMUP_MMS):
    """SPMD program: NA degree-5 atom slots per core (multiple of 8)."""
    nc = bacc.Bacc("TRN2", target_bir_lowering=False, debug=False,
                   enable_asserts=False, num_devices=NCORES)

    # wside: cols 0:128 w0a | 128:256 w0b(rows0-31) | 256:384 w1 |
    # 384:384+2NA napT slots 0,1 | hi5 | lo5 | iw15 | nactT.
    # nap tensor holds slots 2-4.
    NAP0 = 384
    HI0 = 384 + 2 * NA
    WS = HI0 + 384 + NA
    wside_d = nc.dram_tensor("wside", [128, WS], BF, kind="ExternalInput").ap()
    nap_d = nc.dram_tensor("nap", [128, 3, NA], BF, kind="ExternalInput").ap()
    bop_d = nc.dram_tensor("bop", [32, 6, NA], BF, kind="ExternalInput").ap()
    outp = nc.dram_tensor("outp", [128, NA], BF, kind="ExternalOutput")
    outp_ap = outp.ap()

    H = NA - 16  # first (large) output chunk; tiny last chunk for the tail

    with tile.TileContext(nc) as tc:
        with (
            tc.tile_pool(name="w", bufs=1) as wp,
            tc.tile_pool(name="work", bufs=8) as work,
            tc.tile_pool(name="psM", bufs=3, space=bass.MemorySpace.PSUM) as psM,
            tc.tile_pool(name="psA", bufs=2, space=bass.MemorySpace.PSUM) as psA,
        ):
            wside = wp.tile([128, WS], BF, tag="wside")
            nap = wp.tile([128, 3, NA], BF, tag="nap")
            bop = wp.tile([32, 6, NA], BF, tag="bop")

            # ---- input DMAs (need-order; only SP+ACT queues do HWDGE) ----
            nc.sync.dma_start(wside[:, 0:HI0], wside_d[:, 0:HI0])
            nc.scalar.dma_start(bop[:, 0:4, :], bop_d[:, 0:4, :])
            nc.sync.dma_start(nap[:], nap_d[:])
            nc.scalar.dma_start(bop[:, 4:6, :], bop_d[:, 4:6, :])
            nc.sync.dma_start(wside[:, HI0:WS], wside_d[:, HI0:WS])

            w0a = wside[:, 0:128]
            w0b = wside[0:32, 128:256]
            w1 = wside[:, 256:384]

            def napg(g):  # nbr slots for group g
                if g == 0:
                    return wside[:, NAP0:NAP0 + 2 * NA]
                return nap[:, 2 * (g - 1):min(2 * g, 3), :].rearrange(
                    "p a b -> p (a b)")

            hi5 = wside[:, HI0:HI0 + 128]
            lo5 = wside[:, HI0 + 128:HI0 + 256]
            iw15 = wside[:, HI0 + 256:HI0 + 384]
            nact = wside[:, HI0 + 384:WS]

            # ---- PE clock-ramp burst + ACT exp-table prewarm -------------
            wz = wp.tile([128, 512], BF, tag="wz")
            nc.vector.memset(wz[:], 0.0)
            escr = wp.tile([128, 1], F32, tag="escr")
            nc.scalar.activation(escr[:], wz[:, 0:1], AF.Exp)
            if warmup:
                pw = psA.tile([128, 512], F32, tag="ps")
                for _ in range(warmup):
                    nc.tensor.matmul(pw[:], wz[:, 0:128], wz[:],
                                     start=True, stop=True)

            # ---- msg layer 0: 3 groups of 2 edge slots -------------------
            bopv = bop[:].rearrange("p a b -> p (a b)")
            pms = []
            for g in range(3):
                pm = psM.tile([128, 2 * NA], F32, tag="pm")
                nc.tensor.matmul(pm[:], w0b, bopv[:, 2 * g * NA:(2 * g + 2) * NA],
                                 start=True, stop=False)
                w = 2 * NA if g < 2 else NA
                nc.tensor.matmul(pm[:, 0:w], w0a, napg(g),
                                 start=False, stop=True)
                pms.append(pm)

            # elu: exp on ACT, fused combine on DVE (GPSIMD can't read PSUM)
            def elu_tile(pv, out_ap, cols):
                """pv: PSUM f32 [128, cols]; out_ap: SBUF bf16 dest."""
                e = work.tile([128, cols], BF, tag=f"e{cols}")
                nc.scalar.activation(e[:], pv, AF.Exp)
                nc.vector._custom_dve(ELU_OP, out=out_ap, in0=pv,
                                      in1=e[:], s0=1.0, s1=-1.0)

            m0 = [work.tile([128, 2 * NA], BF, tag=f"m0_{g}", name=f"m0_{g}")
                  for g in range(3)]
            for g in range(3):
                elu_tile(pms[g][:], m0[g][:], 2 * NA)

            # ---- msg layer 1 --------------------------------------------
            m1 = wp.tile([128, 6, NA], BF, tag="m1")
            pm2s = []
            for g in range(3):
                pm2 = psM.tile([128, 2 * NA], F32, tag="pm")
                nc.tensor.matmul(pm2[:], w1, m0[g][:], start=True, stop=True)
                pm2s.append(pm2)
            for g in range(3):
                elu_tile(pm2s[g][:],
                         m1[:, 2 * g:2 * g + 2, :].rearrange("p a b -> p (a b)"),
                         2 * NA)

            # ---- inner layer 0 (degree-5 weights, PSUM accumulate) ------
            pi = psA.tile([128, NA], F32, tag="ps")
            nc.tensor.matmul(pi[:], lo5, nact, start=True, stop=False)
            for j in range(6):
                nc.tensor.matmul(pi[:], hi5, m1[:, j, :],
                                 start=False, stop=(j == 5))
            h0 = wp.tile([128, NA], BF, tag="h0")
            elu_tile(pi[:], h0[:], NA)

            # ---- inner layer 1 + output (uneven chunks, two DMA queues;
            # separate PSUM tiles so the chunks have no WAR dep on each
            # other; small chunk first so the big chunk's DMA is the only
            # tail) ----
            obuf = wp.tile([128, NA], BF, tag="obuf")
            po_b = psA.tile([128, NA - H], F32, tag="psb")
            nc.tensor.matmul(po_b[:], iw15, h0[:, H:NA], start=True, stop=True)
            po_a = psA.tile([128, H], F32, tag="ps")
            nc.tensor.matmul(po_a[:], iw15, h0[:, 0:H], start=True, stop=True)
            elu_tile(po_b[:], obuf[:, H:NA], NA - H)
            nc.scalar.dma_start(outp_ap[:, H:NA], obuf[:, H:NA])
            elu_tile(po_a[:], obuf[:, 0:H], H)
            nc.sync.dma_start(outp_ap[:, 0:H], obuf[:, 0:H])

    nc.compile()
    return nc


_CACHE = {}


# --------------------------------------------------------------------------
# host side
# --------------------------------------------------------------------------

def _elu(x):
    return np.where(x > 0, x, np.expm1(np.minimum(x, 0.0)))


def _host_fallback(af, bf, ef, deg, ids, msg_w0, msg_w1, inner_w0, inner_w1):
    """Exact f32 reference for the (few) active atoms with degree < 5.
    af: (N,FA) atoms flat; bf: (N,D,FB); ef: (N,D); ids: flat atom indices."""
    if len(ids) == 0:
        return np.zeros((0, CONV), np.float32)
    mol = ids // M
    e = ef[ids]                                   # (n, D)
    nbr = np.where(e[..., None] >= 0,
                   af[(mol[:, None] * M + np.maximum(e, 0)).ravel()]
                   .reshape(len(ids), D, FA),
                   0.0)
    msg_in = np.concatenate([nbr, bf[ids]], axis=-1)        # (n, D, FA+FB)
    msg = _elu(msg_in @ msg_w0)
    msg = _elu(msg @ msg_w1)
    summed = msg.sum(axis=1)                                # (n, MSG)
    s2 = np.concatenate([summed, af[ids]], axis=-1)         # (n, MSG+FA)
    dg = deg[ids]
    h = _elu(np.einsum('nf,nfc->nc', s2, inner_w0[dg]))
    h = _elu(np.einsum('nc,nce->ne', h, inner_w1[dg]))
    return h.astype(np.float32)


def _prep_core(af, bf, ef, ids, NA):
    """Stage one core's deg-5 atoms (flat ids into af/bf/ef).
    Returns (bop, napf, nact); nap slots 0,1 and nact ride in wside."""
    n = len(ids)
    mol = ids // M
    e = ef[ids]                                   # (n, 6), exactly one -1
    real = e >= 0                                 # (n, 6) 5 True per row
    # pack real edges into slots 0-4, pad bond into slot 5
    order = np.argsort(~real, axis=1, kind="stable")   # real first
    e_p = np.take_along_axis(e, order, axis=1)         # (n,6) col5 = -1
    b_p = np.take_along_axis(bf[ids], order[..., None], axis=1)  # (n,6,FB)

    src = af[(mol[:, None] * M + e_p[:, :5]).ravel()].reshape(n, 5, FA)
    napf = np.zeros((128, 5, NA), np.float32)
    napf[:, :, :n] = src.transpose(2, 1, 0)
    bop = np.zeros((32, 6, NA), BF16)
    bop[:, :, :n] = b_p.transpose(2, 1, 0).astype(BF16)
    nact = np.zeros((128, NA), np.float32)
    nact[:, :n] = af[ids].T
    return bop, napf, nact


def _pack_wside(msg_w0, msg_w1, inner_w0, inner_w1, napf, nact, NA):
    HI0 = 384 + 2 * NA
    ws = np.zeros((128, HI0 + 384 + NA), np.float32)
    ws[:, 0:128] = msg_w0[:128]
    ws[0:32, 128:256] = msg_w0[128:160]
    ws[:, 256:384] = msg_w1
    ws[:, 384:HI0] = napf[:, 0:2, :].reshape(128, 2 * NA)
    ws[:, HI0:HI0 + 128] = inner_w0[5, :128, :]
    ws[:, HI0 + 128:HI0 + 256] = inner_w0[5, 128:, :]
    ws[:, HI0 + 256:HI0 + 384] = inner_w1[5]
    ws[:, HI0 + 384:] = nact
    return ws.astype(BF16)


def kernel(atoms, bonds, edges, msg_w0, msg_w1, inner_w0, inner_w1):
    atoms = np.asarray(atoms, np.float32)
    bonds = np.asarray(bonds, np.float32)
    edges = np.asarray(edges, np.int32)
    msg_w0 = np.asarray(msg_w0, np.float32)
    msg_w1 = np.asarray(msg_w1, np.float32)
    inner_w0 = np.asarray(inner_w0, np.float32)
    inner_w1 = np.asarray(inner_w1, np.float32)

    af = atoms.reshape(B * M, FA)
    bf = bonds.reshape(B * M, D, FB)
    ef = edges.reshape(B * M, D)
    deg = (ef != -1).sum(-1)

    d5 = np.nonzero(deg == 5)[0]
    rest = np.nonzero(deg < 5)[0]

    # balanced round-robin assignment of deg-5 atoms to cores
    per_core = [d5[c::NCORES] for c in range(NCORES)]
    NA = max(16, _roundup(max(len(p) for p in per_core), 8))

    if NA not in _CACHE:
        _CACHE[NA] = build_program(NA)
    nc = _CACHE[NA]

    in_maps = []
    for c in range(NCORES):
        ids = per_core[c]
        bop, napf, nact = _prep_core(af, bf, ef, ids, NA)
        in_maps.append({
            "bop": bop,
            "nap": np.ascontiguousarray(napf[:, 2:5, :]).astype(BF16),
            "wside": _pack_wside(msg_w0, msg_w1, inner_w0, inner_w1,
                                 napf, nact, NA),
        })

    res = bass_utils.run_bass_kernel_spmd(
        nc, in_maps, core_ids=list(range(NCORES)))

    out = np.zeros((B * M, CONV), np.float32)
    for c in range(NCORES):
        ids = per_core[c]
        o = np.asarray(res.results[c]["outp"]).astype(np.float32)  # (128, NA)
        out[ids] = o[:, :len(ids)].T
    out[rest] = _host_fallback(af, bf, ef, deg, rest,
                               msg_w0, msg_w1, inner_w0, inner_w1)
    return out.reshape(B, M, CONV)


# revision 29
# speedup vs baseline: 1.1304x; 1.1304x over previous
"""Trainium2 Bass kernel for nn_NeuralGraphHidden (GNN message passing).

Structure: edges ~ randint(-1, 128) makes ~95.5% of atoms degree 6, whose
outputs are exactly zero (the reference's degree mask covers 0..5 only).  Of
the ~1440 "active" atoms, ~99% are degree 5.  The device handles ONLY the
degree-5 atoms (balanced across the 8 cores, NA~184/core); the handful of
degree<5 atoms are computed exactly on the host in numpy (microseconds).

Everything on device is bf16 (f32 PSUM accumulation): halves DMA vs f32,
LDWEIGHTS gets FWL (2x), and matmuls stream 1 col/cycle at any width.
Per-core device pipeline, with edge slots packed so the 5 real edges occupy
slots 0-4 and the padding slot's bond sits in slot 5 (nbr contribution zero):

  msg0_g  = elu(W0a.T @ nbrT_g + W0b.T @ bondT_g)    3 groups of 2 slots
  msg1_g  = elu(W1.T @ msg0_g)
  h0      = elu(iw0lo5.T @ actT + sum_j iw0hi5.T @ msg1_j)  (PSUM accumulate)
  out     = elu(iw15.T @ h0)                          -> bf16 DMA out

elu(x) = relu(x) + min(exp(x), 1) - 1: exp on the ACT engine (bf16 out), the
combine as one fused custom-DVE op.  An ACT-table prewarm and a PE clock-ramp
matmul burst run during the initial DMA wait.
"""

import sys

if "/opt/trn_rl_repo" not in sys.path:
    sys.path.insert(0, "/opt/trn_rl_repo")

import numpy as np
import ml_dtypes

import concourse.bass as bass
import concourse.bacc as bacc
import concourse.mybir as mybir
import concourse.tile as tile
from concourse import bass_utils

import concourse.dve_ops as dve_ops
from concourse.dve_spec import (Spec, Src0, Src1, C0, C1, Zero, maxx, minn,
                                lower)
from concourse.dve_uop import DveOpSpec


def _make_elu_op():
    """out = relu(in0) + min(in1, c0) + c1  -- with c0=1, c1=-1 and
    in1=exp(in0) this is exactly elu(in0)."""
    name = "ELU_FUSED_ANT"
    for op in dve_ops.OPS:
        if op.name == name:
            return op
    spec = Spec(
        body=maxx(Src0, Zero) + minn(Src1, C0) + C1,
        reference=lambda in0, in1, c0, c1, c2: (
            np.maximum(in0.astype(np.float32), 0)
            + np.minimum(in1.astype(np.float32), c0) + c1),
    )
    idx = dve_ops._CUSTOM_DVE_ROW_BASE + len(dve_ops.OPS)
    shas = {}
    for ver in ("v3", "v4"):
        compiled = DveOpSpec(name=name, opcode=idx, uops=lower(spec, ver=ver),
                             rd1_en=True)
        shas[ver] = compiled.sha(ver)
    op = dve_ops.DveOp(name, spec, subdim=False, uops_sha=shas)
    dve_ops.OPS.append(op)
    dve_ops.CUSTOM_DVE_SPECS[name] = spec
    dve_ops._SUB_OPCODE_FOR_NAME[name] = idx
    return op


ELU_OP = _make_elu_op()

BF16 = ml_dtypes.bfloat16
F32 = mybir.dt.float32
BF = mybir.dt.bfloat16
AF = mybir.ActivationFunctionType
ALU = mybir.AluOpType

B, M, D = 256, 128, 6
FA, FB, MSG, CONV = 128, 32, 128, 128
NCORES = 8

WARMUP_MMS = 5       # PE clock-ramp burst during the initial DMA wait


def _roundup(x, m):
    return (x + m - 1) // m * m


# --------------------------------------------------------------------------
# device program
# --------------------------------------------------------------------------

def build_program(NA, warmup=WARMUP_MMS):
    """SPMD program: NA degree-5 atom slots per core (multiple of 8)."""
    nc = bacc.Bacc("TRN2", target_bir_lowering=False, debug=False,
                   enable_asserts=False, num_devices=NCORES)

    # wside: cols 0:128 w0a | 128:256 w0b(rows0-31) | 256:384 w1 |
    # 384:384+2NA napT slots 0,1 | hi5 | lo5 | iw15 | nactT.
    # nap tensor holds slots 2-4.
    NAP0 = 384
    HI0 = 384 + 2 * NA
    WS = HI0 + 384 + NA
    wside_d = nc.dram_tensor("wside", [128, WS], BF, kind="ExternalInput").ap()
    nap_d = nc.dram_tensor("nap", [128, 3, NA], BF, kind="ExternalInput").ap()
    bop_d = nc.dram_tensor("bop", [32, 6, NA], BF, kind="ExternalInput").ap()
    outp = nc.dram_tensor("outp", [128, NA], BF, kind="ExternalOutput")
    outp_ap = outp.ap()

    H = NA - 16  # first (large) output chunk; tiny last chunk for the tail

    with tile.TileContext(nc) as tc:
        with (
            tc.tile_pool(name="w", bufs=1) as wp,
            tc.tile_pool(name="work", bufs=8) as work,
            tc.tile_pool(name="psM", bufs=3, space=bass.MemorySpace.PSUM) as psM,
            tc.tile_pool(name="psA", bufs=2, space=bass.MemorySpace.PSUM) as psA,
        ):
            wside = wp.tile([128, WS], BF, tag="wside")
            nap = wp.tile([128, 3, NA], BF, tag="nap")
            bop = wp.tile([32, 6, NA], BF, tag="bop")

            # ---- input DMAs (need-order; only SP+ACT queues do HWDGE) ----
            nc.sync.dma_start(wside[:, 0:HI0], wside_d[:, 0:HI0])
            nc.scalar.dma_start(bop[:, 0:2, :], bop_d[:, 0:2, :])
            nc.sync.dma_start(nap[:], nap_d[:])
            nc.scalar.dma_start(bop[:, 2:6, :], bop_d[:, 2:6, :])
            nc.sync.dma_start(wside[:, HI0:WS], wside_d[:, HI0:WS])

            w0a = wside[:, 0:128]
            w0b = wside[0:32, 128:256]
            w1 = wside[:, 256:384]

            def napg(g):  # nbr slots for group g
                if g == 0:
                    return wside[:, NAP0:NAP0 + 2 * NA]
                return nap[:, 2 * (g - 1):min(2 * g, 3), :].rearrange(
                    "p a b -> p (a b)")

            hi5 = wside[:, HI0:HI0 + 128]
            lo5 = wside[:, HI0 + 128:HI0 + 256]
            iw15 = wside[:, HI0 + 256:HI0 + 384]
            nact = wside[:, HI0 + 384:WS]

            # ---- PE clock-ramp burst + ACT exp-table prewarm -------------
            wz = wp.tile([128, 512], BF, tag="wz")
            nc.vector.memset(wz[:], 0.0)
            escr = wp.tile([128, 1], F32, tag="escr")
            nc.scalar.activation(escr[:], wz[:, 0:1], AF.Exp)
            if warmup:
                pw = psA.tile([128, 512], F32, tag="ps")
                for _ in range(warmup):
                    nc.tensor.matmul(pw[:], wz[:, 0:128], wz[:],
                                     start=True, stop=True)

            # ---- msg layer 0: 3 groups of 2 edge slots -------------------
            bopv = bop[:].rearrange("p a b -> p (a b)")
            pms = []
            for g in range(3):
                pm = psM.tile([128, 2 * NA], F32, tag="pm")
                nc.tensor.matmul(pm[:], w0b, bopv[:, 2 * g * NA:(2 * g + 2) * NA],
                                 start=True, stop=False)
                w = 2 * NA if g < 2 else NA
                nc.tensor.matmul(pm[:, 0:w], w0a, napg(g),
                                 start=False, stop=True)
                pms.append(pm)

            # elu: exp on ACT, fused combine on DVE (GPSIMD can't read PSUM)
            def elu_tile(pv, out_ap, cols):
                """pv: PSUM f32 [128, cols]; out_ap: SBUF bf16 dest."""
                e = work.tile([128, cols], BF, tag=f"e{cols}")
                nc.scalar.activation(e[:], pv, AF.Exp)
                nc.vector._custom_dve(ELU_OP, out=out_ap, in0=pv,
                                      in1=e[:], s0=1.0, s1=-1.0)

            m0 = [work.tile([128, 2 * NA], BF, tag=f"m0_{g}", name=f"m0_{g}")
                  for g in range(3)]
            for g in range(3):
                elu_tile(pms[g][:], m0[g][:], 2 * NA)

            # ---- msg layer 1 --------------------------------------------
            m1 = wp.tile([128, 6, NA], BF, tag="m1")
            pm2s = []
            for g in range(3):
                pm2 = psM.tile([128, 2 * NA], F32, tag="pm")
                nc.tensor.matmul(pm2[:], w1, m0[g][:], start=True, stop=True)
                pm2s.append(pm2)
            for g in range(3):
                elu_tile(pm2s[g][:],
                         m1[:, 2 * g:2 * g + 2, :].rearrange("p a b -> p (a b)"),
                         2 * NA)

            # ---- inner layer 0 (degree-5 weights, PSUM accumulate) ------
            pi = psA.tile([128, NA], F32, tag="ps")
            nc.tensor.matmul(pi[:], lo5, nact, start=True, stop=False)
            for j in range(6):
                nc.tensor.matmul(pi[:], hi5, m1[:, j, :],
                                 start=False, stop=(j == 5))
            h0 = wp.tile([128, NA], BF, tag="h0")
            elu_tile(pi[:], h0[:], NA)

            # ---- inner layer 1 + output (uneven chunks, two DMA queues;
            # separate PSUM tiles so chunk 2's matmul has no WAR dep on
            # chunk 1's fuse) ----
            obuf = wp.tile([128, NA], BF, tag="obuf")
            po_a = psA.tile([128, H], F32, tag="ps")
            nc.tensor.matmul(po_a[:], iw15, h0[:, 0:H], start=True, stop=True)
            po_b = psA.tile([128, NA - H], F32, tag="psb")
            nc.tensor.matmul(po_b[:], iw15, h0[:, H:NA], start=True, stop=True)
            elu_tile(po_a[:], obuf[:, 0:H], H)
            nc.sync.dma_start(outp_ap[:, 0:H], obuf[:, 0:H])
            elu_tile(po_b[:], obuf[:, H:NA], NA - H)
            nc.scalar.dma_start(outp_ap[:, H:NA], obuf[:, H:NA])

    nc.compile()
    return nc


_CACHE = {}


# --------------------------------------------------------------------------
# host side
# --------------------------------------------------------------------------

def _elu(x):
    return np.where(x > 0, x, np.expm1(np.minimum(x, 0.0)))


def _host_fallback(af, bf, ef, deg, ids, msg_w0, msg_w1, inner_w0, inner_w1):
    """Exact f32 reference for the (few) active atoms with degree < 5.
    af: (N,FA) atoms flat; bf: (N,D,FB); ef: (N,D); ids: flat atom indices."""
    if len(ids) == 0:
        return np.zeros((0, CONV), np.float32)
    mol = ids // M
    e = ef[ids]                                   # (n, D)
    nbr = np.where(e[..., None] >= 0,
                   af[(mol[:, None] * M + np.maximum(e, 0)).ravel()]
                   .reshape(len(ids), D, FA),
                   0.0)
    msg_in = np.concatenate([nbr, bf[ids]], axis=-1)        # (n, D, FA+FB)
    msg = _elu(msg_in @ msg_w0)
    msg = _elu(msg @ msg_w1)
    summed = msg.sum(axis=1)                                # (n, MSG)
    s2 = np.concatenate([summed, af[ids]], axis=-1)         # (n, MSG+FA)
    dg = deg[ids]
    h = _elu(np.einsum('nf,nfc->nc', s2, inner_w0[dg]))
    h = _elu(np.einsum('nc,nce->ne', h, inner_w1[dg]))
    return h.astype(np.float32)


def _prep_core(af, bf, ef, ids, NA):
    """Stage one core's deg-5 atoms (flat ids into af/bf/ef).
    Returns (bop, napf, nact); nap slots 0,1 and nact ride in wside."""
    n = len(ids)
    mol = ids // M
    e = ef[ids]                                   # (n, 6), exactly one -1
    real = e >= 0                                 # (n, 6) 5 True per row
    # pack real edges into slots 0-4, pad bond into slot 5
    order = np.argsort(~real, axis=1, kind="stable")   # real first
    e_p = np.take_along_axis(e, order, axis=1)         # (n,6) col5 = -1
    b_p = np.take_along_axis(bf[ids], order[..., None], axis=1)  # (n,6,FB)

    src = af[(mol[:, None] * M + e_p[:, :5]).ravel()].reshape(n, 5, FA)
    napf = np.zeros((128, 5, NA), np.float32)
    napf[:, :, :n] = src.transpose(2, 1, 0)
    bop = np.zeros((32, 6, NA), BF16)
    bop[:, :, :n] = b_p.transpose(2, 1, 0).astype(BF16)
    nact = np.zeros((128, NA), np.float32)
    nact[:, :n] = af[ids].T
    return bop, napf, nact


def _pack_wside(msg_w0, msg_w1, inner_w0, inner_w1, napf, nact, NA):
    HI0 = 384 + 2 * NA
    ws = np.zeros((128, HI0 + 384 + NA), np.float32)
    ws[:, 0:128] = msg_w0[:128]
    ws[0:32, 128:256] = msg_w0[128:160]
    ws[:, 256:384] = msg_w1
    ws[:, 384:HI0] = napf[:, 0:2, :].reshape(128, 2 * NA)
    ws[:, HI0:HI0 + 128] = inner_w0[5, :128, :]
    ws[:, HI0 + 128:HI0 + 256] = inner_w0[5, 128:, :]
    ws[:, HI0 + 256:HI0 + 384] = inner_w1[5]
    ws[:, HI0 + 384:] = nact
    return ws.astype(BF16)


def kernel(atoms, bonds, edges, msg_w0, msg_w1, inner_w0, inner_w1):
    atoms = np.asarray(atoms, np.float32)
    bonds = np.asarray(bonds, np.float32)
    edges = np.asarray(edges, np.int32)
    msg_w0 = np.asarray(msg_w0, np.float32)
    msg_w1 = np.asarray(msg_w1, np.float32)
    inner_w0 = np.asarray(inner_w0, np.float32)
    inner_w1 = np.asarray(inner_w1, np.float32)

    af = atoms.reshape(B * M, FA)
    bf = bonds.reshape(B * M, D, FB)
    ef = edges.reshape(B * M, D)
    deg = (ef != -1).sum(-1)

    d5 = np.nonzero(deg == 5)[0]
    rest = np.nonzero(deg < 5)[0]

    # balanced round-robin assignment of deg-5 atoms to cores
    per_core = [d5[c::NCORES] for c in range(NCORES)]
    NA = max(16, _roundup(max(len(p) for p in per_core), 8))

    if NA not in _CACHE:
        _CACHE[NA] = build_program(NA)
    nc = _CACHE[NA]

    in_maps = []
    for c in range(NCORES):
        ids = per_core[c]
        bop, napf, nact = _prep_core(af, bf, ef, ids, NA)
        in_maps.append({
            "bop": bop,
            "nap": np.ascontiguousarray(napf[:, 2:5, :]).astype(BF16),
            "wside": _pack_wside(msg_w0, msg_w1, inner_w0, inner_w1,
                                 napf, nact, NA),
        })

    res = bass_utils.run_bass_kernel_spmd(
        nc, in_maps, core_ids=list(range(NCORES)))

    out = np.zeros((B * M, CONV), np.float32)
    for c in range(NCORES):
        ids = per_core[c]
        o = np.asarray(res.results[c]["outp"]).astype(np.float32)  # (128, NA)
        out[ids] = o[:, :len(ids)].T
    out[rest] = _host_fallback(af, bf, ef, deg, rest,
                               msg_w0, msg_w1, inner_w0, inner_w1)
    return out.reshape(B, M, CONV)
